# revision 15
# baseline (speedup 1.0000x reference)
"""Distributed KNN retrieval kernel for Trainium2 (8 NeuronCores).

reference semantics:
    qn = queries / ||queries||
    scores = (qn @ keys.T) / 0.25           # keys pre-normalized
    topv, topidx = top_k(scores, 16)
    topw = softmax(topv) (eps in denom)
    retrieved = sum_k topw[:,k] * values[topidx[:,k]]
    returns (retrieved, topw, topidx)

Sharding: keys/values row-sharded across 8 cores; queries replicated.
Each core computes local scores + local top-16, cores AllGather the
8*16 candidates, reduce to global top-16, softmax, masked local values
gather, AllReduce of partial weighted sums.
"""

import numpy as np

import concourse.bass as bass
import concourse.bacc as bacc
import concourse.mybir as mybir
import concourse.tile as tile
from concourse.bass import IndirectOffsetOnAxis
from concourse.bass_utils import run_bass_kernel_spmd
from concourse.masks import make_identity

F32 = mybir.dt.float32
U32 = mybir.dt.uint32
AX = mybir.AxisListType
ALU = mybir.AluOpType
ACTF = mybir.ActivationFunctionType

B = 512           # queries
KD = 128          # key dim
VD = 64           # value dim
TOPK = 16
N = 500000        # memory slots
CORES = 8
NS = N // CORES   # 62500 keys per core
TEMP = 0.25
EPS = 1e-8

NEG = -1e30


def build_nc(nw: int, ns: int, merge_on_device: bool = True,
             debug_outs: bool = False) -> bass.Bass:
    """Build the SPMD program (identical on all cores).

    nw: number of 1024-key windows per core (padded size = nw*1024)
    ns: real (unpadded) keys per core
    """
    npad = nw * 1024
    assert nw % 2 == 0, "DMA macro covers 2 windows"
    ncand = nw * 8  # candidates per query per core

    nc = bacc.Bacc("TRN2", debug=False, num_devices=CORES)

    # ---- I/O ----
    queries = nc.dram_tensor("queries", [B, KD], F32, kind="ExternalInput").ap()
    keysT = nc.dram_tensor("keysT", [KD, npad], F32, kind="ExternalInput").ap()
    vals = nc.dram_tensor("vals", [ns, VD], F32, kind="ExternalInput").ap()
    # iota rows (0..n-1 replicated on each partition) for select-by-position
    iota_cand = nc.dram_tensor("iota_cand", [128, ncand], F32,
                               kind="ExternalInput").ap()
    iota128 = nc.dram_tensor("iota128", [128, CORES * TOPK], F32,
                             kind="ExternalInput").ap()
    # coff[p] = core_off (replicated)
    coff = nc.dram_tensor("coff", [128, 1], U32, kind="ExternalInput").ap()
    coff_f = nc.dram_tensor("coff_f", [128, 1], F32, kind="ExternalInput").ap()

    if merge_on_device:
        out_ret = nc.dram_tensor("retrieved", [B, VD], F32, kind="ExternalOutput").ap()
        out_topw = nc.dram_tensor("topw", [B, TOPK], F32, kind="ExternalOutput").ap()
        out_topi = nc.dram_tensor("topidx", [B, TOPK], U32, kind="ExternalOutput").ap()
        if debug_outs:
            out_lidx = nc.dram_tensor("dbg_lidx", [B, TOPK], U32,
                                      kind="ExternalOutput").ap()
            out_vrows = nc.dram_tensor("dbg_vrows", [B, TOPK * VD], F32,
                                       kind="ExternalOutput").ap()
            out_part = nc.dram_tensor("dbg_part", [B, VD], F32,
                                      kind="ExternalOutput").ap()
    else:
        out_lv = nc.dram_tensor("localv", [B, TOPK], F32, kind="ExternalOutput").ap()
        out_li = nc.dram_tensor("localidx", [B, TOPK], U32, kind="ExternalOutput").ap()

    NCH = B // 128  # query chunks

    with tile.TileContext(nc) as tc:
        with (
            tc.tile_pool(name="const", bufs=1) as cpool,
            tc.tile_pool(name="qprep", bufs=2) as qpool,
            tc.tile_pool(name="keys", bufs=3) as kpool,
            tc.tile_pool(name="cand", bufs=1) as candpool,
            tc.tile_pool(name="work", bufs=2) as wpool,
            tc.tile_pool(name="psum", bufs=3, space="PSUM") as psum,
            tc.tile_pool(name="psq", bufs=1, space="PSUM") as psq,
            tc.tile_pool(name="dram", bufs=2, space="DRAM") as dram,
        ):
            ident0 = cpool.tile([128, 128], F32, tag="ident0")
            make_identity(nc, ident0[:])
            # route through DVE so PE transpose needs only one wait sem
            ident = cpool.tile([128, 128], F32, tag="ident")
            nc.vector.tensor_copy(out=ident[:], in_=ident0[:])

            iota_cand_sb = cpool.tile([128, ncand], F32, tag="iotac")
            nc.sync.dma_start(out=iota_cand_sb[:], in_=iota_cand)
            iota128_sb = cpool.tile([128, CORES * TOPK], F32, tag="iota128")
            nc.sync.dma_start(out=iota128_sb[:], in_=iota128)
            coff_sb = cpool.tile([128, 1], U32, tag="coff")
            nc.sync.dma_start(out=coff_sb[:], in_=coff)
            coff_f_sb = cpool.tile([128, 1], F32, tag="cofff")
            nc.sync.dma_start(out=coff_f_sb[:], in_=coff_f)
            sh3 = cpool.tile([128, 1], U32, tag="sh3")
            nc.vector.memset(sh3[:], 3)
            sh10 = cpool.tile([128, 1], U32, tag="sh10")
            nc.vector.memset(sh10[:], 10)

            # ---- query prep: normalize, fold 1/TEMP, transpose ----
            qnT = []
            for c in range(NCH):
                q_c = qpool.tile([128, KD], F32, tag="q")
                nc.sync.dma_start(out=q_c[:], in_=queries[c * 128:(c + 1) * 128, :])
                sq = qpool.tile([128, KD], F32, tag="sq")
                nc.vector.tensor_tensor(out=sq[:], in0=q_c[:], in1=q_c[:], op=ALU.mult)
                ss = qpool.tile([128, 1], F32, tag="ss")
                nc.vector.reduce_sum(out=ss[:], in_=sq[:], axis=AX.X)
                rinv = qpool.tile([128, 1], F32, tag="rinv")
                nc.vector.reciprocal(out=rinv[:], in_=ss[:])
                rn = qpool.tile([128, 1], F32, tag="rn")
                # sqrt((1/ss) / TEMP^2) = (1/TEMP) / sqrt(ss)
                nc.scalar.activation(out=rn[:], in_=rinv[:], func=ACTF.Sqrt,
                                     scale=float(1.0 / (TEMP * TEMP)))
                qn = qpool.tile([128, KD], F32, tag="qn")
                nc.vector.tensor_scalar_mul(qn[:], q_c[:], rn[:])
                pt = psq.tile([128, 128], F32, tag="ptrans")
                nc.tensor.transpose(out=pt[:], in_=qn[:], identity=ident[:])
                qnT_c = cpool.tile([128, 128], F32, tag=f"qnT{c}")
                nc.vector.tensor_copy(out=qnT_c[:], in_=pt[:])
                qnT.append(qnT_c)

            # ---- main scan: matmul + per-window top-8 ----
            cand_v = [candpool.tile([128, ncand], F32, name=f"cv{c}", tag=f"cv{c}")
                      for c in range(NCH)]
            cand_i = [candpool.tile([128, ncand], U32, name=f"ci{c}", tag=f"ci{c}")
                      for c in range(NCH)]

            for m in range(nw // 2):  # 2048-key DMA macro tiles
                kt = kpool.tile([128, 2048], F32, tag="kt")
                nc.gpsimd.dma_start(out=kt[:], in_=keysT[:, m * 2048:(m + 1) * 2048])
                for h in range(2):
                    w = m * 2 + h
                    for c in range(NCH):
                        ps = psum.tile([128, 1024], F32, tag="ps")
                        nc.tensor.matmul(out=ps[:, 0:512], lhsT=qnT[c][:],
                                         rhs=kt[:, h * 1024:h * 1024 + 512],
                                         start=True, stop=True)
                        nc.tensor.matmul(out=ps[:, 512:1024], lhsT=qnT[c][:],
                                         rhs=kt[:, h * 1024 + 512:h * 1024 + 1024],
                                         start=True, stop=True)
                        nc.vector.max(out=cand_v[c][:, w * 8:(w + 1) * 8], in_=ps[:])
                        nc.vector.max_index(out=cand_i[c][:, w * 8:(w + 1) * 8],
                                            in_max=cand_v[c][:, w * 8:(w + 1) * 8],
                                            in_values=ps[:])

            # ---- per-chunk local top-16 + global index recovery ----
            lv_tiles, li_tiles = [], []
            for c in range(NCH):
                sc = wpool.tile([128, ncand], F32, tag="sc")
                topv16 = wpool.tile([128, TOPK], F32, tag="topv16")
                nc.vector.max(out=topv16[:, 0:8], in_=cand_v[c][:])
                nc.vector.match_replace(out=sc[:], in_to_replace=topv16[:, 0:8],
                                        in_values=cand_v[c][:], imm_value=NEG)
                nc.vector.max(out=topv16[:, 8:16], in_=sc[:])
                pos = wpool.tile([128, TOPK], U32, tag="pos")
                nc.vector.max_index(out=pos[:, 0:8], in_max=topv16[:, 0:8],
                                    in_values=cand_v[c][:])
                nc.vector.max_index(out=pos[:, 8:16], in_max=topv16[:, 8:16],
                                    in_values=sc[:])

                # j16[p,k] = cand_i[c][p, pos[p,k]] via (iota==pos)*data sum
                cand_if = wpool.tile([128, ncand], F32, tag="candif")
                nc.vector.tensor_copy(out=cand_if[:], in_=cand_i[c][:])
                posf = wpool.tile([128, TOPK], F32, tag="posf")
                nc.vector.tensor_copy(out=posf[:], in_=pos[:])
                gidx_f = wpool.tile([128, TOPK], F32, tag="gidxf")
                eqscr = wpool.tile([128, ncand], F32, tag="eqscr")
                for k in range(TOPK):
                    nc.vector.scalar_tensor_tensor(
                        out=eqscr[:], in0=iota_cand_sb[:],
                        scalar=posf[:, k:k + 1], in1=cand_if[:],
                        op0=ALU.is_equal, op1=ALU.mult,
                        accum_out=gidx_f[:, k:k + 1])
                # window base: (pos>>3)<<10 ; gidx = j16 + base + core_off
                j16u = wpool.tile([128, TOPK], U32, tag="j16u")
                nc.vector.tensor_copy(out=j16u[:], in_=gidx_f[:])
                wbase = wpool.tile([128, TOPK], U32, tag="wbase")
                nc.vector.tensor_tensor(out=wbase[:], in0=pos[:],
                                        in1=sh3[:].to_broadcast([128, TOPK]),
                                        op=ALU.logical_shift_right)
                nc.vector.tensor_tensor(out=wbase[:], in0=wbase[:],
                                        in1=sh10[:].to_broadcast([128, TOPK]),
                                        op=ALU.logical_shift_left)
                gidx = wpool.tile([128, TOPK], U32, tag="gidx")
                nc.vector.tensor_tensor(out=gidx[:], in0=j16u[:], in1=wbase[:],
                                        op=ALU.add)
                nc.vector.tensor_tensor(out=gidx[:], in0=gidx[:],
                                        in1=coff_sb[:].to_broadcast([128, TOPK]),
                                        op=ALU.add)
                lv_tiles.append(topv16)
                li_tiles.append(gidx)

            if not merge_on_device:
                for c in range(NCH):
                    nc.sync.dma_start(out=out_lv[c * 128:(c + 1) * 128, :],
                                      in_=lv_tiles[c][:])
                    nc.sync.dma_start(out=out_li[c * 128:(c + 1) * 128, :],
                                      in_=li_tiles[c][:])
            else:
                # ---- all-gather candidates across cores ----
                lv_b = dram.tile([B, TOPK], F32, tag="lvb")
                li_b = dram.tile([B, TOPK], U32, tag="lib")
                for c in range(NCH):
                    nc.sync.dma_start(out=lv_b[c * 128:(c + 1) * 128, :],
                                      in_=lv_tiles[c][:])
                    nc.sync.dma_start(out=li_b[c * 128:(c + 1) * 128, :],
                                      in_=li_tiles[c][:])
                allv_b = dram.tile([CORES * B, TOPK], F32, tag="allvb")
                alli_b = dram.tile([CORES * B, TOPK], U32, tag="allib")
                nc.gpsimd.collective_compute(
                    "AllGather", ALU.bypass,
                    replica_groups=[list(range(CORES))],
                    ins=[lv_b.opt()], outs=[allv_b.opt()])
                nc.gpsimd.collective_compute(
                    "AllGather", ALU.bypass,
                    replica_groups=[list(range(CORES))],
                    ins=[li_b.opt()], outs=[alli_b.opt()])

                ret_b = dram.tile([B, VD], F32, tag="retb")

                CW = CORES * TOPK  # 128 merged candidates per query
                for c in range(NCH):
                    # load merged candidate rows [q, (core k)]
                    mv = wpool.tile([128, CW], F32, tag="mv")
                    mi = wpool.tile([128, CW], U32, tag="mi")
                    src_v = allv_b[:].rearrange("(cc q) k -> q cc k", cc=CORES)
                    src_i = alli_b[:].rearrange("(cc q) k -> q cc k", cc=CORES)
                    nc.sync.dma_start(
                        out=mv[:].rearrange("p (cc k) -> p cc k", cc=CORES),
                        in_=src_v[c * 128:(c + 1) * 128, :, :])
                    nc.sync.dma_start(
                        out=mi[:].rearrange("p (cc k) -> p cc k", cc=CORES),
                        in_=src_i[c * 128:(c + 1) * 128, :, :])

                    msc = wpool.tile([128, CW], F32, tag="msc")
                    topg = wpool.tile([128, TOPK], F32, tag="topg")
                    nc.vector.max(out=topg[:, 0:8], in_=mv[:])
                    nc.vector.match_replace(out=msc[:], in_to_replace=topg[:, 0:8],
                                            in_values=mv[:], imm_value=NEG)
                    nc.vector.max(out=topg[:, 8:16], in_=msc[:])
                    pg = wpool.tile([128, TOPK], U32, tag="pg")
                    nc.vector.max_index(out=pg[:, 0:8], in_max=topg[:, 0:8],
                                        in_values=mv[:])
                    nc.vector.max_index(out=pg[:, 8:16], in_max=topg[:, 8:16],
                                        in_values=msc[:])

                    mif = wpool.tile([128, CW], F32, tag="mif")
                    nc.vector.tensor_copy(out=mif[:], in_=mi[:])
                    pgf = wpool.tile([128, TOPK], F32, tag="pgf")
                    nc.vector.tensor_copy(out=pgf[:], in_=pg[:])
                    gidxg_f = wpool.tile([128, TOPK], F32, tag="gidxgf")
                    eqs2 = wpool.tile([128, CW], F32, tag="eqs2")
                    for k in range(TOPK):
                        nc.vector.scalar_tensor_tensor(
                            out=eqs2[:], in0=iota128_sb[:],
                            scalar=pgf[:, k:k + 1], in1=mif[:],
                            op0=ALU.is_equal, op1=ALU.mult,
                            accum_out=gidxg_f[:, k:k + 1])
                    gidxg = wpool.tile([128, TOPK], U32, tag="gidxg")
                    nc.vector.tensor_copy(out=gidxg[:], in_=gidxg_f[:])
                    nc.sync.dma_start(out=out_topi[c * 128:(c + 1) * 128, :],
                                      in_=gidxg[:])

                    # softmax over topg
                    rmax = wpool.tile([128, 1], F32, tag="rmax")
                    nc.vector.reduce_max(out=rmax[:], in_=topg[:], axis=AX.X)
                    nrmax = wpool.tile([128, 1], F32, tag="nrmax")
                    nc.vector.tensor_scalar_mul(nrmax[:], rmax[:], -1.0)
                    ex = wpool.tile([128, TOPK], F32, tag="ex")
                    nc.scalar.activation(out=ex[:], in_=topg[:], func=ACTF.Exp,
                                         bias=nrmax[:], scale=1.0)
                    sm = wpool.tile([128, 1], F32, tag="sm")
                    nc.vector.reduce_sum(out=sm[:], in_=ex[:], axis=AX.X)
                    sme = wpool.tile([128, 1], F32, tag="sme")
                    nc.vector.tensor_scalar_add(sme[:], sm[:], float(EPS))
                    rs = wpool.tile([128, 1], F32, tag="rs")
                    nc.vector.reciprocal(out=rs[:], in_=sme[:])
                    topw = wpool.tile([128, TOPK], F32, tag="topw")
                    nc.vector.tensor_scalar_mul(topw[:], ex[:], rs[:])
                    nc.sync.dma_start(out=out_topw[c * 128:(c + 1) * 128, :],
                                      in_=topw[:])

                    # masked local values gather + partial weighted sum
                    lidx = wpool.tile([128, TOPK], U32, tag="lidx")
                    nc.vector.tensor_tensor(out=lidx[:], in0=gidxg[:],
                                            in1=coff_sb[:].to_broadcast([128, TOPK]),
                                            op=ALU.subtract)
                    vrows = wpool.tile([128, TOPK * VD], F32, tag="vrows")
                    nc.vector.memset(vrows[:], 0.0)
                    for k in range(TOPK):
                        nc.gpsimd.indirect_dma_start(
                            out=vrows[:, k * VD:(k + 1) * VD], out_offset=None,
                            in_=vals,
                            in_offset=IndirectOffsetOnAxis(ap=lidx[:, k:k + 1],
                                                           axis=0),
                            bounds_check=ns - 1, oob_is_err=False)
                    # zero weights for candidates below this core's range
                    # (u32 subtract saturates at 0 -> they'd gather row 0)
                    ownedf = wpool.tile([128, TOPK], F32, tag="ownedf")
                    nc.vector.tensor_scalar(out=ownedf[:], in0=gidxg_f[:],
                                            scalar1=coff_f_sb[:], scalar2=None,
                                            op0=ALU.is_ge)
                    weff = wpool.tile([128, TOPK], F32, tag="weff")
                    nc.vector.tensor_tensor(out=weff[:], in0=topw[:],
                                            in1=ownedf[:], op=ALU.mult)
                    acc = wpool.tile([128, VD], F32, tag="acc")
                    nc.vector.memset(acc[:], 0.0)
                    for j in range(TOPK):
                        nc.vector.scalar_tensor_tensor(
                            out=acc[:], in0=vrows[:, j * VD:(j + 1) * VD],
                            scalar=weff[:, j:j + 1], in1=acc[:],
                            op0=ALU.mult, op1=ALU.add)
                    nc.sync.dma_start(out=ret_b[c * 128:(c + 1) * 128, :], in_=acc[:])
                    if debug_outs:
                        nc.sync.dma_start(out=out_lidx[c * 128:(c + 1) * 128, :],
                                          in_=lidx[:])
                        nc.sync.dma_start(out=out_vrows[c * 128:(c + 1) * 128, :],
                                          in_=vrows[:])
                        nc.sync.dma_start(out=out_part[c * 128:(c + 1) * 128, :],
                                          in_=acc[:])

                ret_o = dram.tile([B, VD], F32, tag="reto")
                nc.gpsimd.collective_compute(
                    "AllReduce", ALU.add,
                    replica_groups=[list(range(CORES))],
                    ins=[ret_b.opt()], outs=[ret_o.opt()])
                nc.sync.dma_start(out=out_ret[:, :], in_=ret_o[:])

    nc.compile()
    return nc


def make_in_maps(queries: np.ndarray, keys: np.ndarray, values: np.ndarray,
                 nw: int, ns: int) -> list[dict[str, np.ndarray]]:
    npad = nw * 1024
    ncand = nw * 8
    qf = np.ascontiguousarray(queries, dtype=np.float32)
    in_maps = []
    iota_cand = np.broadcast_to(np.arange(ncand, dtype=np.float32),
                                (128, ncand)).copy()
    iota128 = np.broadcast_to(np.arange(CORES * TOPK, dtype=np.float32),
                              (128, CORES * TOPK)).copy()
    for c in range(CORES):
        off = c * ns
        kT = np.zeros((KD, npad), dtype=np.float32)
        kT[:, :ns] = keys[off:off + ns].T
        coff = np.full((128, 1), off, dtype=np.uint32)
        coff_f = np.full((128, 1), off, dtype=np.float32)
        in_maps.append({
            "queries": qf,
            "keysT": np.ascontiguousarray(kT),
            "vals": np.ascontiguousarray(values[off:off + ns], dtype=np.float32),
            "iota_cand": iota_cand,
            "iota128": iota128,
            "coff": coff,
            "coff_f": coff_f,
        })
    return in_maps


_NC_CACHE: dict = {}


def kernel(queries, keys, values, topk):
    assert int(topk) == TOPK
    queries = np.asarray(queries, dtype=np.float32)
    keys = np.asarray(keys, dtype=np.float32)
    values = np.asarray(values, dtype=np.float32)

    nw = 64  # 64 windows * 1024 = 65536 padded slots per core
    key = ("full", nw)
    if key not in _NC_CACHE:
        _NC_CACHE[key] = build_nc(nw=nw, ns=NS, merge_on_device=True)
    nc = _NC_CACHE[key]

    in_maps = make_in_maps(queries, keys, values, nw=nw, ns=NS)
    res = run_bass_kernel_spmd(nc, in_maps, core_ids=list(range(CORES)))
    r0 = res.results[0]
    retrieved = np.asarray(r0["retrieved"], dtype=np.float32)
    topw = np.asarray(r0["topw"], dtype=np.float32)
    topidx = np.asarray(r0["topidx"]).astype(np.int32)
    return retrieved, topw, topidx


# revision 21
# speedup vs baseline: 15.7652x; 15.7652x over previous
"""Distributed KNN retrieval kernel for Trainium2 (8 NeuronCores).

reference semantics:
    qn = queries / ||queries||
    scores = (qn @ keys.T) / 0.25           # keys pre-normalized
    topv, topidx = top_k(scores, 16)
    topw = softmax(topv) (eps in denom)
    retrieved = sum_k topw[:,k] * values[topidx[:,k]]
    returns (retrieved, topw, topidx)

Sharding: keys/values row-sharded across 8 cores; queries replicated.
Each core computes local scores + local top-16, cores AllGather the
8*16 candidates, reduce to global top-16, softmax, masked local values
gather, AllReduce of partial weighted sums.
"""

import numpy as np

import concourse.bass as bass
import concourse.bacc as bacc
import concourse.mybir as mybir
import concourse.tile as tile
from concourse.bass import IndirectOffsetOnAxis
from concourse.bass_utils import run_bass_kernel_spmd
from concourse.masks import make_identity

F32 = mybir.dt.float32
F16 = mybir.dt.float16
U32 = mybir.dt.uint32
AX = mybir.AxisListType
ALU = mybir.AluOpType
ACTF = mybir.ActivationFunctionType

B = 512           # queries
KD = 128          # key dim
VD = 64           # value dim
TOPK = 16
N = 500000        # memory slots
CORES = 8
NS = N // CORES   # 62500 keys per core
TEMP = 0.25
EPS = 1e-8

NEG = -1e30


def build_nc(nw: int, ns: int, merge_on_device: bool = True,
             debug_outs: bool = False, fake_merge: bool = False,
             deferred_idx: bool = False) -> bass.Bass:
    """Build the SPMD program (identical on all cores).

    nw: number of 1024-key windows per core (padded size = nw*1024)
    ns: real (unpadded) keys per core
    """
    npad = nw * 1024
    assert nw % 2 == 0, "DMA macro covers 2 windows"
    ncand = nw * 8  # candidates per query per core

    nc = bacc.Bacc("TRN2", debug=False, num_devices=CORES)

    # ---- I/O ----
    queries = nc.dram_tensor("queries", [B, KD], F32, kind="ExternalInput").ap()
    keysT = nc.dram_tensor("keysT", [KD, npad], F32, kind="ExternalInput").ap()
    vals = nc.dram_tensor("vals", [ns, VD], F32, kind="ExternalInput").ap()
    # iota rows (0..n-1 replicated on each partition) for select-by-position
    iota_cand = nc.dram_tensor("iota_cand", [128, ncand], F32,
                               kind="ExternalInput").ap()
    iota128 = nc.dram_tensor("iota128", [128, CORES * TOPK], F32,
                             kind="ExternalInput").ap()
    # coff[p] = core_off (replicated)
    coff = nc.dram_tensor("coff", [128, 1], U32, kind="ExternalInput").ap()
    if fake_merge:
        allv_in = nc.dram_tensor("allv_in", [CORES * B, TOPK], F32,
                                 kind="ExternalInput").ap()
        alli_in = nc.dram_tensor("alli_in", [CORES * B, TOPK], U32,
                                 kind="ExternalInput").ap()
    coff_f = nc.dram_tensor("coff_f", [128, 1], F32, kind="ExternalInput").ap()
    prow_npad = nc.dram_tensor("prow_npad", [128, 1], U32, kind="ExternalInput").ap()

    if merge_on_device:
        out_ret = nc.dram_tensor("retrieved", [B, VD], F32, kind="ExternalOutput").ap()
        out_topw = nc.dram_tensor("topw", [B, TOPK], F32, kind="ExternalOutput").ap()
        out_topi = nc.dram_tensor("topidx", [B, TOPK], U32, kind="ExternalOutput").ap()
        if debug_outs:
            out_lidx = nc.dram_tensor("dbg_lidx", [B, TOPK], U32,
                                      kind="ExternalOutput").ap()
            out_vrows = nc.dram_tensor("dbg_vrows", [B, TOPK * VD], F32,
                                       kind="ExternalOutput").ap()
            out_part = nc.dram_tensor("dbg_part", [B, VD], F32,
                                      kind="ExternalOutput").ap()
    else:
        out_lv = nc.dram_tensor("localv", [B, TOPK], F32, kind="ExternalOutput").ap()
        out_li = nc.dram_tensor("localidx", [B, TOPK], U32, kind="ExternalOutput").ap()

    NCH = B // 128  # query chunks

    with tile.TileContext(nc) as tc:
        with (
            tc.tile_pool(name="const", bufs=1) as cpool,
            tc.tile_pool(name="qprep", bufs=2) as qpool,
            tc.tile_pool(name="keys", bufs=3) as kpool,
            tc.tile_pool(name="cand", bufs=1) as candpool,
            tc.tile_pool(name="work", bufs=2) as wpool,
            tc.tile_pool(name="psum", bufs=3, space="PSUM") as psum,
            tc.tile_pool(name="psq", bufs=1, space="PSUM") as psq,
            tc.tile_pool(name="dram", bufs=2, space="DRAM") as dram,
        ):
            ident0 = cpool.tile([128, 128], F32, tag="ident0")
            make_identity(nc, ident0[:])
            # route through DVE so PE transpose needs only one wait sem
            ident = cpool.tile([128, 128], F32, tag="ident")
            nc.vector.tensor_copy(out=ident[:], in_=ident0[:])

            iota_cand_sb = cpool.tile([128, ncand], F32, tag="iotac")
            nc.sync.dma_start(out=iota_cand_sb[:], in_=iota_cand)
            iota128_sb = cpool.tile([128, CORES * TOPK], F32, tag="iota128")
            nc.sync.dma_start(out=iota128_sb[:], in_=iota128)
            coff_sb = cpool.tile([128, 1], U32, tag="coff")
            nc.sync.dma_start(out=coff_sb[:], in_=coff)
            coff_f_sb = cpool.tile([128, 1], F32, tag="cofff")
            nc.sync.dma_start(out=coff_f_sb[:], in_=coff_f)
            prow_npad_sb = cpool.tile([128, 1], U32, tag="prownp")
            nc.sync.dma_start(out=prow_npad_sb[:], in_=prow_npad)
            sh3 = cpool.tile([128, 1], U32, tag="sh3")
            nc.vector.memset(sh3[:], 3)
            sh10 = cpool.tile([128, 1], U32, tag="sh10")
            nc.vector.memset(sh10[:], 10)

            # ---- query prep: normalize, fold 1/TEMP, transpose ----
            qnT = []
            for c in range(NCH):
                q_c = qpool.tile([128, KD], F32, tag="q")
                nc.sync.dma_start(out=q_c[:], in_=queries[c * 128:(c + 1) * 128, :])
                sq = qpool.tile([128, KD], F32, tag="sq")
                nc.vector.tensor_tensor(out=sq[:], in0=q_c[:], in1=q_c[:], op=ALU.mult)
                ss = qpool.tile([128, 1], F32, tag="ss")
                nc.vector.reduce_sum(out=ss[:], in_=sq[:], axis=AX.X)
                rinv = qpool.tile([128, 1], F32, tag="rinv")
                nc.vector.reciprocal(out=rinv[:], in_=ss[:])
                rn = qpool.tile([128, 1], F32, tag="rn")
                # sqrt((1/ss) / TEMP^2) = (1/TEMP) / sqrt(ss)
                nc.scalar.activation(out=rn[:], in_=rinv[:], func=ACTF.Sqrt,
                                     scale=float(1.0 / (TEMP * TEMP)))
                qn = qpool.tile([128, KD], F32, tag="qn")
                nc.vector.tensor_scalar_mul(qn[:], q_c[:], rn[:])
                pt = psq.tile([128, 128], F32, tag="ptrans")
                nc.tensor.transpose(out=pt[:], in_=qn[:], identity=ident[:])
                qnT_c = cpool.tile([128, 128], F32, tag=f"qnT{c}")
                nc.vector.tensor_copy(out=qnT_c[:], in_=pt[:])
                qnT.append(qnT_c)

            # ---- main scan: matmul + per-window top-8 ----
            cand_v = [candpool.tile([128, ncand], F32, name=f"cv{c}", tag=f"cv{c}")
                      for c in range(NCH)]
            if deferred_idx:
                caches = [dram.tile([128 * npad, 1], F16, name=f"cache{c}",
                                    tag=f"cache{c}", bufs=1) for c in range(NCH)]
            else:
                cand_i = [candpool.tile([128, ncand], U32, name=f"ci{c}",
                                        tag=f"ci{c}") for c in range(NCH)]

            for m in range(nw // 2):  # 2048-key DMA macro tiles
                kt = kpool.tile([128, 2048], F32, tag="kt")
                nc.gpsimd.dma_start(out=kt[:], in_=keysT[:, m * 2048:(m + 1) * 2048])
                for h in range(2):
                    w = m * 2 + h
                    for c in range(NCH):
                        ps = psum.tile([128, 1024], F32, tag="ps")
                        nc.tensor.matmul(out=ps[:, 0:512], lhsT=qnT[c][:],
                                         rhs=kt[:, h * 1024:h * 1024 + 512],
                                         start=True, stop=True)
                        nc.tensor.matmul(out=ps[:, 512:1024], lhsT=qnT[c][:],
                                         rhs=kt[:, h * 1024 + 512:h * 1024 + 1024],
                                         start=True, stop=True)
                        nc.vector.max(out=cand_v[c][:, w * 8:(w + 1) * 8], in_=ps[:])
                        if deferred_idx:
                            st = wpool.tile([128, 1024], F16, tag="stage16")
                            nc.scalar.copy(out=st[:], in_=ps[:])
                            nc.sync.dma_start(
                                out=caches[c][:, 0].rearrange(
                                    "(p f) -> p f", p=128)[:, w * 1024:(w + 1) * 1024],
                                in_=st[:])
                        else:
                            nc.vector.max_index(
                                out=cand_i[c][:, w * 8:(w + 1) * 8],
                                in_max=cand_v[c][:, w * 8:(w + 1) * 8],
                                in_values=ps[:])

            # ---- per-chunk local top-16 + global index recovery ----
            lv_tiles, li_tiles = [], []
            for c in range(NCH):
                sc = wpool.tile([128, ncand], F32, tag="sc")
                topv16 = wpool.tile([128, TOPK], F32, tag="topv16")
                nc.vector.max(out=topv16[:, 0:8], in_=cand_v[c][:])
                nc.vector.match_replace(out=sc[:], in_to_replace=topv16[:, 0:8],
                                        in_values=cand_v[c][:], imm_value=NEG)
                nc.vector.max(out=topv16[:, 8:16], in_=sc[:])
                pos = wpool.tile([128, TOPK], U32, tag="pos")
                nc.vector.max_index(out=pos[:, 0:8], in_max=topv16[:, 0:8],
                                    in_values=cand_v[c][:])
                nc.vector.max_index(out=pos[:, 8:16], in_max=topv16[:, 8:16],
                                    in_values=sc[:])

                # window base: (pos>>3)<<10 ; gidx = j16 + base + core_off
                j16u = wpool.tile([128, TOPK], U32, tag="j16u")
                if deferred_idx:
                    pass  # j16u filled below from cache search
                else:
                    # j16[p,k] = cand_i[c][p, pos[p,k]] via (iota==pos)*data sum
                    cand_if = wpool.tile([128, ncand], F32, tag="candif")
                    nc.vector.tensor_copy(out=cand_if[:], in_=cand_i[c][:])
                    posf = wpool.tile([128, TOPK], F32, tag="posf")
                    nc.vector.tensor_copy(out=posf[:], in_=pos[:])
                    gidx_f = wpool.tile([128, TOPK], F32, tag="gidxf")
                    eqscr = wpool.tile([128, ncand], F32, tag="eqscr")
                    for k in range(TOPK):
                        nc.vector.scalar_tensor_tensor(
                            out=eqscr[:], in0=iota_cand_sb[:],
                            scalar=posf[:, k:k + 1], in1=cand_if[:],
                            op0=ALU.is_equal, op1=ALU.mult,
                            accum_out=gidx_f[:, k:k + 1])
                    nc.vector.tensor_copy(out=j16u[:], in_=gidx_f[:])
                wbase = wpool.tile([128, TOPK], U32, tag="wbase")
                nc.vector.tensor_tensor(out=wbase[:], in0=pos[:],
                                        in1=sh3[:].to_broadcast([128, TOPK]),
                                        op=ALU.logical_shift_right)
                nc.vector.tensor_tensor(out=wbase[:], in0=wbase[:],
                                        in1=sh10[:].to_broadcast([128, TOPK]),
                                        op=ALU.logical_shift_left)
                if deferred_idx:
                    # search each finalist's 1024-wide window in the fp16 cache
                    offs_all = wpool.tile([128, TOPK], U32, tag="offsall")
                    nc.vector.tensor_tensor(
                        out=offs_all[:], in0=wbase[:],
                        in1=prow_npad_sb[:].to_broadcast([128, TOPK]), op=ALU.add)
                    v16h = wpool.tile([128, TOPK], F16, tag="v16h")
                    nc.scalar.copy(out=v16h[:], in_=topv16[:])
                    j8scr = wpool.tile([128, 8], U32, tag="j8scr")
                    for k in range(TOPK):
                        gath = wpool.tile([128, 1024], F16, tag="gath")
                        nc.gpsimd.indirect_dma_start(
                            out=gath[:], out_offset=None,
                            in_=caches[c][:],
                            in_offset=IndirectOffsetOnAxis(
                                ap=offs_all[:, k:k + 1], axis=0))
                        nc.vector.max_index(
                            out=j8scr[:],
                            in_max=v16h[:, k:k + 1].to_broadcast([128, 8]),
                            in_values=gath[:])
                        nc.vector.tensor_copy(out=j16u[:, k:k + 1],
                                              in_=j8scr[:, 0:1])
                gidx = wpool.tile([128, TOPK], U32, tag="gidx")
                nc.vector.tensor_tensor(out=gidx[:], in0=j16u[:], in1=wbase[:],
                                        op=ALU.add)
                nc.vector.tensor_tensor(out=gidx[:], in0=gidx[:],
                                        in1=coff_sb[:].to_broadcast([128, TOPK]),
                                        op=ALU.add)
                lv_tiles.append(topv16)
                li_tiles.append(gidx)

            if not merge_on_device:
                for c in range(NCH):
                    nc.sync.dma_start(out=out_lv[c * 128:(c + 1) * 128, :],
                                      in_=lv_tiles[c][:])
                    nc.sync.dma_start(out=out_li[c * 128:(c + 1) * 128, :],
                                      in_=li_tiles[c][:])
            else:
                # ---- all-gather candidates across cores ----
                lv_b = dram.tile([B, TOPK], F32, tag="lvb")
                li_b = dram.tile([B, TOPK], U32, tag="lib")
                for c in range(NCH):
                    nc.sync.dma_start(out=lv_b[c * 128:(c + 1) * 128, :],
                                      in_=lv_tiles[c][:])
                    nc.sync.dma_start(out=li_b[c * 128:(c + 1) * 128, :],
                                      in_=li_tiles[c][:])
                if fake_merge:
                    allv_b = allv_in
                    alli_b = alli_in
                else:
                    allv_b = dram.tile([CORES * B, TOPK], F32, tag="allvb")
                    alli_b = dram.tile([CORES * B, TOPK], U32, tag="allib")
                    nc.gpsimd.collective_compute(
                        "AllGather", ALU.bypass,
                        replica_groups=[list(range(CORES))],
                        ins=[lv_b.opt()], outs=[allv_b.opt()])
                    nc.gpsimd.collective_compute(
                        "AllGather", ALU.bypass,
                        replica_groups=[list(range(CORES))],
                        ins=[li_b.opt()], outs=[alli_b.opt()])

                ret_b = dram.tile([B, VD], F32, tag="retb")

                CW = CORES * TOPK  # 128 merged candidates per query
                for c in range(NCH):
                    # load merged candidate rows [q, (core k)]
                    mv = wpool.tile([128, CW], F32, tag="mv")
                    mi = wpool.tile([128, CW], U32, tag="mi")
                    src_v = (allv_b if fake_merge else allv_b[:]).rearrange(
                        "(cc q) k -> q cc k", cc=CORES)
                    src_i = (alli_b if fake_merge else alli_b[:]).rearrange(
                        "(cc q) k -> q cc k", cc=CORES)
                    nc.sync.dma_start(
                        out=mv[:].rearrange("p (cc k) -> p cc k", cc=CORES),
                        in_=src_v[c * 128:(c + 1) * 128, :, :])
                    nc.sync.dma_start(
                        out=mi[:].rearrange("p (cc k) -> p cc k", cc=CORES),
                        in_=src_i[c * 128:(c + 1) * 128, :, :])

                    msc = wpool.tile([128, CW], F32, tag="msc")
                    topg = wpool.tile([128, TOPK], F32, tag="topg")
                    nc.vector.max(out=topg[:, 0:8], in_=mv[:])
                    nc.vector.match_replace(out=msc[:], in_to_replace=topg[:, 0:8],
                                            in_values=mv[:], imm_value=NEG)
                    nc.vector.max(out=topg[:, 8:16], in_=msc[:])
                    pg = wpool.tile([128, TOPK], U32, tag="pg")
                    nc.vector.max_index(out=pg[:, 0:8], in_max=topg[:, 0:8],
                                        in_values=mv[:])
                    nc.vector.max_index(out=pg[:, 8:16], in_max=topg[:, 8:16],
                                        in_values=msc[:])

                    mif = wpool.tile([128, CW], F32, tag="mif")
                    nc.vector.tensor_copy(out=mif[:], in_=mi[:])
                    pgf = wpool.tile([128, TOPK], F32, tag="pgf")
                    nc.vector.tensor_copy(out=pgf[:], in_=pg[:])
                    gidxg_f = wpool.tile([128, TOPK], F32, tag="gidxgf")
                    eqs2 = wpool.tile([128, CW], F32, tag="eqs2")
                    for k in range(TOPK):
                        nc.vector.scalar_tensor_tensor(
                            out=eqs2[:], in0=iota128_sb[:],
                            scalar=pgf[:, k:k + 1], in1=mif[:],
                            op0=ALU.is_equal, op1=ALU.mult,
                            accum_out=gidxg_f[:, k:k + 1])
                    gidxg = wpool.tile([128, TOPK], U32, tag="gidxg")
                    nc.vector.tensor_copy(out=gidxg[:], in_=gidxg_f[:])
                    nc.sync.dma_start(out=out_topi[c * 128:(c + 1) * 128, :],
                                      in_=gidxg[:])

                    # softmax over topg
                    rmax = wpool.tile([128, 1], F32, tag="rmax")
                    nc.vector.reduce_max(out=rmax[:], in_=topg[:], axis=AX.X)
                    nrmax = wpool.tile([128, 1], F32, tag="nrmax")
                    nc.vector.tensor_scalar_mul(nrmax[:], rmax[:], -1.0)
                    ex = wpool.tile([128, TOPK], F32, tag="ex")
                    nc.scalar.activation(out=ex[:], in_=topg[:], func=ACTF.Exp,
                                         bias=nrmax[:], scale=1.0)
                    sm = wpool.tile([128, 1], F32, tag="sm")
                    nc.vector.reduce_sum(out=sm[:], in_=ex[:], axis=AX.X)
                    sme = wpool.tile([128, 1], F32, tag="sme")
                    nc.vector.tensor_scalar_add(sme[:], sm[:], float(EPS))
                    rs = wpool.tile([128, 1], F32, tag="rs")
                    nc.vector.reciprocal(out=rs[:], in_=sme[:])
                    topw = wpool.tile([128, TOPK], F32, tag="topw")
                    nc.vector.tensor_scalar_mul(topw[:], ex[:], rs[:])
                    nc.sync.dma_start(out=out_topw[c * 128:(c + 1) * 128, :],
                                      in_=topw[:])

                    # masked local values gather + partial weighted sum
                    lidx = wpool.tile([128, TOPK], U32, tag="lidx")
                    nc.vector.tensor_tensor(out=lidx[:], in0=gidxg[:],
                                            in1=coff_sb[:].to_broadcast([128, TOPK]),
                                            op=ALU.subtract)
                    vrows = wpool.tile([128, TOPK * VD], F32, tag="vrows")
                    nc.vector.memset(vrows[:], 0.0)
                    for k in range(TOPK):
                        nc.gpsimd.indirect_dma_start(
                            out=vrows[:, k * VD:(k + 1) * VD], out_offset=None,
                            in_=vals,
                            in_offset=IndirectOffsetOnAxis(ap=lidx[:, k:k + 1],
                                                           axis=0),
                            bounds_check=ns - 1, oob_is_err=False)
                    # zero weights for candidates below this core's range
                    # (u32 subtract saturates at 0 -> they'd gather row 0)
                    ownedf = wpool.tile([128, TOPK], F32, tag="ownedf")
                    nc.vector.tensor_scalar(out=ownedf[:], in0=gidxg_f[:],
                                            scalar1=coff_f_sb[:], scalar2=None,
                                            op0=ALU.is_ge)
                    weff = wpool.tile([128, TOPK], F32, tag="weff")
                    nc.vector.tensor_tensor(out=weff[:], in0=topw[:],
                                            in1=ownedf[:], op=ALU.mult)
                    acc = wpool.tile([128, VD], F32, tag="acc")
                    nc.vector.memset(acc[:], 0.0)
                    for j in range(TOPK):
                        nc.vector.scalar_tensor_tensor(
                            out=acc[:], in0=vrows[:, j * VD:(j + 1) * VD],
                            scalar=weff[:, j:j + 1], in1=acc[:],
                            op0=ALU.mult, op1=ALU.add)
                    nc.sync.dma_start(out=ret_b[c * 128:(c + 1) * 128, :], in_=acc[:])
                    if debug_outs:
                        nc.sync.dma_start(out=out_lidx[c * 128:(c + 1) * 128, :],
                                          in_=lidx[:])
                        nc.sync.dma_start(out=out_vrows[c * 128:(c + 1) * 128, :],
                                          in_=vrows[:])
                        nc.sync.dma_start(out=out_part[c * 128:(c + 1) * 128, :],
                                          in_=acc[:])

                if fake_merge:
                    nc.sync.dma_start(out=out_ret[:, :], in_=ret_b[:])
                else:
                    ret_o = dram.tile([B, VD], F32, tag="reto")
                    nc.gpsimd.collective_compute(
                        "AllReduce", ALU.add,
                        replica_groups=[list(range(CORES))],
                        ins=[ret_b.opt()], outs=[ret_o.opt()])
                    nc.sync.dma_start(out=out_ret[:, :], in_=ret_o[:])

    nc.compile()
    return nc


def make_in_maps(queries: np.ndarray, keys: np.ndarray, values: np.ndarray,
                 nw: int, ns: int) -> list[dict[str, np.ndarray]]:
    npad = nw * 1024
    ncand = nw * 8
    qf = np.ascontiguousarray(queries, dtype=np.float32)
    in_maps = []
    iota_cand = np.broadcast_to(np.arange(ncand, dtype=np.float32),
                                (128, ncand)).copy()
    iota128 = np.broadcast_to(np.arange(CORES * TOPK, dtype=np.float32),
                              (128, CORES * TOPK)).copy()
    for c in range(CORES):
        off = c * ns
        kT = np.zeros((KD, npad), dtype=np.float32)
        kT[:, :ns] = keys[off:off + ns].T
        coff = np.full((128, 1), off, dtype=np.uint32)
        coff_f = np.full((128, 1), off, dtype=np.float32)
        in_maps.append({
            "queries": qf,
            "keysT": np.ascontiguousarray(kT),
            "vals": np.ascontiguousarray(values[off:off + ns], dtype=np.float32),
            "iota_cand": iota_cand,
            "iota128": iota128,
            "coff": coff,
            "coff_f": coff_f,
            "prow_npad": (np.arange(128, dtype=np.uint32) * npad).reshape(128, 1),
        })
    return in_maps


_NC_CACHE: dict = {}


def kernel(queries, keys, values, topk):
    assert int(topk) == TOPK
    queries = np.asarray(queries, dtype=np.float32)
    keys = np.asarray(keys, dtype=np.float32)
    values = np.asarray(values, dtype=np.float32)

    nw = 64  # 64 windows * 1024 = 65536 padded slots per core
    key = ("full", nw)
    if key not in _NC_CACHE:
        _NC_CACHE[key] = build_nc(nw=nw, ns=NS, merge_on_device=True)
    nc = _NC_CACHE[key]

    in_maps = make_in_maps(queries, keys, values, nw=nw, ns=NS)
    res = run_bass_kernel_spmd(nc, in_maps, core_ids=list(range(CORES)))
    r0 = res.results[0]
    retrieved = np.asarray(r0["retrieved"], dtype=np.float32)
    topw = np.asarray(r0["topw"], dtype=np.float32)
    topidx = np.asarray(r0["topidx"]).astype(np.int32)
    return retrieved, topw, topidx
